# revision 9
# baseline (speedup 1.0000x reference)
import os
import sys

import ml_dtypes
import numpy as np

for p in ("/opt/trn_rl_repo",):
    if p not in sys.path:
        sys.path.insert(0, p)

import concourse.bass as bass  # noqa: E402
import concourse.tile as tile  # noqa: E402
from concourse import bacc, mybir  # noqa: E402
from concourse.bass_utils import run_bass_kernel_spmd  # noqa: E402

B, N, D = 128, 512, 512
NCORES = 8
BPC = B // NCORES  # 16 batch items per core
F32 = mybir.dt.float32
BF16 = mybir.dt.bfloat16

LAST_RESULTS = None


def _hadamard(n: int) -> np.ndarray:
    H = np.array([[1.0]], dtype=np.float32)
    base = np.array([[1.0, 1.0], [1.0, -1.0]], dtype=np.float32)
    while H.shape[0] < n:
        H = np.kron(H, base)
    return H


def _build():
    nc = bacc.Bacc("TRN2", target_bir_lowering=False, debug=False)
    # x/y as [BPC, 128, 2048] bf16: same bytes as [BPC, 512, 512], partition
    # p holds rows 4p..4p+3 (column block k of 512 = row 4p+k).
    x_d = nc.dram_tensor("x", [BPC, 128, 4 * D], BF16, kind="ExternalInput").ap()
    # h128[p, q] = H128[p, q] (Sylvester Hadamard 128)
    h128_d = nc.dram_tensor("h128", [128, 128], BF16, kind="ExternalInput").ap()
    # hs[c, dt*512 + e] = H512[dt*128+c, e] / 512
    hs_d = nc.dram_tensor("hs", [128, 4 * N], BF16, kind="ExternalInput").ap()
    y_d = nc.dram_tensor("y", [BPC, 128, 4 * D], BF16, kind="ExternalOutput").ap()

    with tile.TileContext(nc) as tc:
        with (
            tc.tile_pool(name="const", bufs=1) as const_pool,
            tc.tile_pool(name="xp", bufs=3) as x_pool,
            tc.tile_pool(name="pm", bufs=2) as pm_pool,
            tc.tile_pool(name="tp", bufs=2) as t_pool,
            tc.tile_pool(name="yp", bufs=3) as y_pool,
            tc.tile_pool(name="ps", bufs=8, space="PSUM") as psum_pool,
        ):
            h128_sb = const_pool.tile([128, 128], BF16, tag="h128")
            nc.sync.dma_start(h128_sb[:], h128_d[:])
            hs_sb = const_pool.tile([128, 4 * N], BF16, tag="hs")
            nc.sync.dma_start(hs_sb[:], hs_d[:])

            for b0 in range(0, BPC, 2):
                # Load a pair of slices as one 1 MiB DMA: xt col block s
                # (0..1) holds slice b0+s.
                xt = x_pool.tile([128, 2 * 4 * D], BF16)
                nc.sync.dma_start(
                    xt[:].rearrange("p (s j) -> p s j", s=2),
                    x_d[b0 : b0 + 2].transpose([1, 0, 2]),
                )
                yt = y_pool.tile([128, 2 * 4 * D], BF16)

                for s in range(2):
                    xo = s * 4 * D
                    yo = s * 4 * D
                    # ---- Pass 1 (left transform), kron H512 = H4 (x) H128:
                    # V_k[c, dt*128+q] = sum_p x[4p+k, dt*128+c] H128[p, q]
                    vk = [
                        psum_pool.tile([128, N], F32, name=f"vk{k}", tag="ps")
                        for k in range(4)
                    ]
                    for k in range(4):
                        for dt_ in range(4):
                            nc.tensor.matmul(
                                vk[k][:, dt_ * 128 : (dt_ + 1) * 128],
                                xt[:, xo + k * D + dt_ * 128 : xo + k * D + dt_ * 128 + 128],
                                h128_sb[:],
                                start=True,
                                stop=True,
                            )
                    # Radix-2 butterfly over k (H4 combine):
                    # t_l = sum_k H4[k,l] V_k;  tt block l = t_l
                    # (TT can read at most one PSUM operand, so V1/V3 are
                    # first staged to SBUF by ScalarE.)
                    vs = pm_pool.tile([128, 2 * N], F32, name="vs", tag="vs")
                    nc.scalar.copy(vs[:, 0:N], vk[1][:])
                    nc.scalar.copy(vs[:, N : 2 * N], vk[3][:])
                    pm = pm_pool.tile([128, 4 * N], BF16)
                    nc.vector.tensor_add(pm[:, 0 * N : 1 * N], vk[0][:], vs[:, 0:N])
                    nc.vector.tensor_sub(pm[:, 1 * N : 2 * N], vk[0][:], vs[:, 0:N])
                    nc.vector.tensor_add(pm[:, 2 * N : 3 * N], vk[2][:], vs[:, N : 2 * N])
                    nc.vector.tensor_sub(pm[:, 3 * N : 4 * N], vk[2][:], vs[:, N : 2 * N])
                    tt = t_pool.tile([128, 4 * N], BF16)
                    P, M, Q, R = (pm[:, i * N : (i + 1) * N] for i in range(4))
                    nc.gpsimd.tensor_add(tt[:, 0 * N : 1 * N], P, Q)
                    nc.gpsimd.tensor_add(tt[:, 1 * N : 2 * N], M, R)
                    nc.gpsimd.tensor_sub(tt[:, 2 * N : 3 * N], P, Q)
                    nc.gpsimd.tensor_sub(tt[:, 3 * N : 4 * N], M, R)

                    # ---- Pass 2 (right transform), full H512:
                    # y[4p+k2, e] = sum_d tT[d, 4p+k2] (H512/512)[d, e]
                    for k2 in range(4):
                        ps = psum_pool.tile([128, D], F32, tag="ps")
                        for dt_ in range(4):
                            nc.tensor.matmul(
                                ps[:],
                                tt[:, k2 * N + dt_ * 128 : k2 * N + dt_ * 128 + 128],
                                hs_sb[:, dt_ * D : (dt_ + 1) * D],
                                start=(dt_ == 0),
                                stop=(dt_ == 3),
                            )
                        nc.scalar.copy(yt[:, yo + k2 * D : yo + (k2 + 1) * D], ps[:])

                nc.sync.dma_start(
                    y_d[b0 : b0 + 2].transpose([1, 0, 2]),
                    yt[:].rearrange("p (s j) -> p s j", s=2),
                )

    nc.compile()
    return nc


_NC = None


def kernel(x: np.ndarray) -> np.ndarray:
    global _NC, LAST_RESULTS
    if _NC is None:
        _NC = _build()
    x = (
        np.ascontiguousarray(np.asarray(x), dtype=np.float32)
        .astype(ml_dtypes.bfloat16)
        .reshape(NCORES, BPC, 128, 4 * D)
    )
    H = _hadamard(N)
    h128 = np.ascontiguousarray(_hadamard(128)).astype(ml_dtypes.bfloat16)
    hs = np.ascontiguousarray(
        H.reshape(4, 128, N).transpose(1, 0, 2).reshape(128, 4 * N)
        / np.float32(512.0)
    ).astype(ml_dtypes.bfloat16)
    in_maps = [{"x": x[i], "h128": h128, "hs": hs} for i in range(NCORES)]
    trace = os.environ.get("KERNEL_TRACE", "") == "1"
    res = run_bass_kernel_spmd(_NC, in_maps, list(range(NCORES)), trace=trace)
    LAST_RESULTS = res
    out = np.stack([np.asarray(r["y"]) for r in res.results], axis=0)
    return out.reshape(B, N, D).astype(np.float32)


# revision 10
# speedup vs baseline: 2.0128x; 2.0128x over previous
import os
import sys

import ml_dtypes
import numpy as np

for p in ("/opt/trn_rl_repo",):
    if p not in sys.path:
        sys.path.insert(0, p)

import concourse.bass as bass  # noqa: E402
import concourse.tile as tile  # noqa: E402
from concourse import bacc, mybir  # noqa: E402
from concourse.bass_utils import run_bass_kernel_spmd  # noqa: E402

B, N, D = 128, 512, 512
NCORES = 8
BPC = B // NCORES  # 16 batch items per core
F32 = mybir.dt.float32
BF16 = mybir.dt.bfloat16

LAST_RESULTS = None


def _hadamard(n: int) -> np.ndarray:
    H = np.array([[1.0]], dtype=np.float32)
    base = np.array([[1.0, 1.0], [1.0, -1.0]], dtype=np.float32)
    while H.shape[0] < n:
        H = np.kron(H, base)
    return H


def _build():
    nc = bacc.Bacc("TRN2", target_bir_lowering=False, debug=False)
    # x/y as [BPC, 128, 2048] bf16: same bytes as [BPC, 512, 512]; partition
    # p holds rows 4p..4p+3 (column block k of 512 = row 4p+k).
    x_d = nc.dram_tensor("x", [BPC, 128, 4 * D], BF16, kind="ExternalInput").ap()
    h128_d = nc.dram_tensor("h128", [128, 128], BF16, kind="ExternalInput").ap()
    # hs[c, dt*512 + e] = H512[dt*128+c, e] / 512
    hs_d = nc.dram_tensor("hs", [128, 4 * N], BF16, kind="ExternalInput").ap()
    y_d = nc.dram_tensor("y", [BPC, 128, 4 * D], BF16, kind="ExternalOutput").ap()

    with tile.TileContext(nc) as tc:
        with (
            tc.tile_pool(name="const", bufs=1) as const_pool,
            tc.tile_pool(name="xp", bufs=3) as x_pool,
            tc.tile_pool(name="xm", bufs=2) as xm_pool,
            tc.tile_pool(name="xc", bufs=3) as xc_pool,
            tc.tile_pool(name="tp", bufs=3) as t_pool,
            tc.tile_pool(name="yp", bufs=3) as y_pool,
            tc.tile_pool(name="ps", bufs=4, space="PSUM") as psum_pool,
        ):
            h128_sb = const_pool.tile([128, 128], BF16, tag="h128")
            nc.sync.dma_start(h128_sb[:], h128_d[:])
            hs_sb = const_pool.tile([128, 4 * N], BF16, tag="hs")
            nc.sync.dma_start(hs_sb[:], hs_d[:])

            W = 4 * D  # 2048, one slice's width
            xcs = {}  # slice -> xc tile (+ column offset)
            tts = {}  # slice -> tt tile
            yts = {}  # pair -> yt tile
            pend = []  # queue of slices whose pass-2 is not yet emitted

            def pair2(ap_tile, s2):
                # [128, 2, W] view of one pair tile, s2 selects the slice col
                return ap_tile[:].rearrange("p (s j) -> p s j", s=2)[:, s2]

            def emit_butterfly(b0):
                # one 1 MiB DMA for the slice pair, then the H4 combine on x:
                # xc_l = sum_k H4[k,l] x_k, done per-pair with [2,512] APs
                xt = x_pool.tile([128, 2 * W], BF16, name="xt")
                nc.sync.dma_start(
                    xt[:].rearrange("p (s j) -> p s j", s=2),
                    x_d[b0 : b0 + 2].transpose([1, 0, 2]),
                )
                xm = xm_pool.tile([128, 2 * W], BF16, name="xm")
                xc = xc_pool.tile([128, 2 * W], BF16, name="xc")

                def blk(t, k):
                    # [128, 2, 512] AP: block k of both slices in the pair
                    return t[:].rearrange("p (s j) -> p s j", s=2)[
                        :, :, k * D : (k + 1) * D
                    ]

                nc.vector.tensor_add(blk(xm, 0), blk(xt, 0), blk(xt, 1))
                nc.vector.tensor_sub(blk(xm, 1), blk(xt, 0), blk(xt, 1))
                nc.vector.tensor_add(blk(xm, 2), blk(xt, 2), blk(xt, 3))
                nc.vector.tensor_sub(blk(xm, 3), blk(xt, 2), blk(xt, 3))
                nc.vector.tensor_add(blk(xc, 0), blk(xm, 0), blk(xm, 2))
                nc.vector.tensor_add(blk(xc, 1), blk(xm, 1), blk(xm, 3))
                nc.gpsimd.tensor_sub(blk(xc, 2), blk(xm, 0), blk(xm, 2))
                nc.gpsimd.tensor_sub(blk(xc, 3), blk(xm, 1), blk(xm, 3))
                xcs[b0] = (xc, 0)
                xcs[b0 + 1] = (xc, W)
                yt = y_pool.tile([128, 2 * W], BF16, name="yt")
                yts[b0] = yt
                yts[b0 + 1] = yt

            def emit_stage_a(s):
                # t_l[c, dt*128+q] = sum_p xc_l[p, dt*128+c] H128[p, q]
                xc, xo = xcs[s]
                tps = [
                    psum_pool.tile([128, 2 * N], F32, name=f"tps{h}", tag="ps")
                    for h in range(2)
                ]
                for l in range(4):
                    for dt_ in range(4):
                        nc.tensor.matmul(
                            tps[l // 2][
                                :, (l % 2) * N + dt_ * 128 : (l % 2) * N + dt_ * 128 + 128
                            ],
                            xc[:, xo + l * D + dt_ * 128 : xo + l * D + dt_ * 128 + 128],
                            h128_sb[:],
                            start=True,
                            stop=True,
                        )
                tt = t_pool.tile([128, 4 * N], BF16, name="tt")
                nc.scalar.copy(tt[:, 0 : 2 * N], tps[0][:])
                nc.scalar.copy(tt[:, 2 * N : 4 * N], tps[1][:])
                tts[s] = tt

            def emit_pass2(s):
                # y[4p+k2, e] = sum_d t_k2[d-part] (H512/512)[d, e]
                tt = tts.pop(s)
                yt = yts.pop(s)
                yo = (s % 2) * W
                pps = [
                    psum_pool.tile([128, 2 * D], F32, name=f"pps{h}", tag="ps")
                    for h in range(2)
                ]
                for k2 in range(4):
                    for dt_ in range(4):
                        nc.tensor.matmul(
                            pps[k2 // 2][:, (k2 % 2) * D : (k2 % 2 + 1) * D],
                            tt[:, k2 * N + dt_ * 128 : k2 * N + dt_ * 128 + 128],
                            hs_sb[:, dt_ * D : (dt_ + 1) * D],
                            start=(dt_ == 0),
                            stop=(dt_ == 3),
                        )
                nc.scalar.copy(yt[:, yo : yo + 2 * D], pps[0][:])
                nc.vector.tensor_copy(yt[:, yo + 2 * D : yo + 4 * D], pps[1][:])
                if s % 2 == 1:
                    b0 = s - 1
                    nc.sync.dma_start(
                        y_d[b0 : b0 + 2].transpose([1, 0, 2]),
                        yt[:].rearrange("p (s j) -> p s j", s=2),
                    )

            # software pipeline: stage_a(s) ... pass2(s-1) interleaved so the
            # PE always has an independent matmul burst queued
            for s in range(BPC):
                if s % 2 == 0:
                    emit_butterfly(s)
                emit_stage_a(s)
                if pend:
                    emit_pass2(pend.pop(0))
                pend.append(s)
            while pend:
                emit_pass2(pend.pop(0))

    nc.compile()
    return nc


_NC = None


def kernel(x: np.ndarray) -> np.ndarray:
    global _NC, LAST_RESULTS
    if _NC is None:
        _NC = _build()
    x = (
        np.ascontiguousarray(np.asarray(x), dtype=np.float32)
        .astype(ml_dtypes.bfloat16)
        .reshape(NCORES, BPC, 128, 4 * D)
    )
    H = _hadamard(N)
    h128 = np.ascontiguousarray(_hadamard(128)).astype(ml_dtypes.bfloat16)
    hs = np.ascontiguousarray(
        H.reshape(4, 128, N).transpose(1, 0, 2).reshape(128, 4 * N)
        / np.float32(512.0)
    ).astype(ml_dtypes.bfloat16)
    in_maps = [{"x": x[i], "h128": h128, "hs": hs} for i in range(NCORES)]
    trace = os.environ.get("KERNEL_TRACE", "") == "1"
    res = run_bass_kernel_spmd(_NC, in_maps, list(range(NCORES)), trace=trace)
    LAST_RESULTS = res
    out = np.stack([np.asarray(r["y"]) for r in res.results], axis=0)
    return out.reshape(B, N, D).astype(np.float32)
